# revision 5
# baseline (speedup 1.0000x reference)
"""AffCoeffToMatrix TRN2 kernel.

For each batch element (B = 2,000,000):
  R = rodrigues(rotat), U = rodrigues(scal_dir), D = exp(scal)
  M = R @ (U @ diag(D) @ U^T);  out = [M | trans]  -> [B, 3, 4] f32

Sharding: pure batch-parallel over 8 NeuronCores (no communication).
Layout on core: batch spread over [128 partitions x F free]; interleaved
raw loads; f32 scalar chain (ln/exp for sqrt+recip, Sin with compare+STT
range wrap); fp16 planar matrix phase to hit DVE 2x mode; ScalarE (ACT)
carries transcendentals, deinterleave and output-interleave copies.
"""
import math
import sys

for _p in ("/opt/trn_rl_repo", "/root/.axon_site/_ro/trn_rl_repo"):
    if _p not in sys.path:
        sys.path.append(_p)

import numpy as np

import concourse.bass as bass
import concourse.mybir as mybir
import concourse.tile as tile

F32 = mybir.dt.float32
F16 = mybir.dt.float16
AF = mybir.ActivationFunctionType
OP = mybir.AluOpType
PI = math.pi

# ---- hardcoded problem geometry ----
B = 2_000_000
N_CORES = 8
P = 128
F = 328            # free-dim elements per tile
T = 6              # tiles per core
L = F * T          # 1968 elements per partition lane
E = P * L          # 251,904 elements per core
BPAD = N_CORES * E # 2,015,232

MAT_DT = F16       # dtype of the matrix-build phase
OPT_S_DIAG_ACT = True   # S diagonal via ACT Square instead of 3 DVE muls
OPT_M_PLANAR = True     # assemble M in fp16 planes; ACT interleaves to f32 out


def _split_multi_waits(nc, limit=1, drain_limit=0):
    """This container's walrus cannot encode >1 sync-wait per instruction
    (Drain: none at all). Spill extras onto same-engine NOPs."""
    for b in nc.main_func.blocks:
        new = []
        for ins in b.instructions:
            si = getattr(ins, "sync_info", None)
            waits = list(si.on_wait) if (si is not None and si.on_wait) else []
            lim = drain_limit if isinstance(ins, mybir.InstDrain) else limit
            if len(waits) > lim:
                keep, spill = waits[:lim], waits[lim:]
                for w in spill:
                    nop = mybir.InstNoOp(
                        name=nc.get_next_instruction_name(),
                        sync_info=mybir.SyncInfo(on_wait=[w], on_update=[]),
                        bass_nofuse=True,
                        engine=ins.engine,
                    )
                    nc.register_instruction(nop)
                    new.append(nop)
                ins.sync_info = mybir.SyncInfo(
                    on_wait=keep, on_update=list(si.on_update or [])
                )
            new.append(ins)
        b.instructions[:] = new


def build_module(F=F, T=T, mat_dt=MAT_DT, loop_rep=None):
    nc = bass.Bass()
    E_ = P * F * T
    rot = nc.dram_tensor("rotat", [E_, 3], F32, kind="ExternalInput")
    sd = nc.dram_tensor("scal_dir", [E_, 3], F32, kind="ExternalInput")
    sc = nc.dram_tensor("scal", [E_, 3], F32, kind="ExternalInput")
    tr = nc.dram_tensor("trans", [E_, 3], F32, kind="ExternalInput")
    out = nc.dram_tensor("out", [E_, 12], F32, kind="ExternalOutput")

    rotv = rot[:].rearrange("(t p f) c -> t p (f c)", t=T, p=P)
    sdv = sd[:].rearrange("(t p f) c -> t p (f c)", t=T, p=P)
    scv = sc[:].rearrange("(t p f) c -> t p (f c)", t=T, p=P)
    trv = tr[:].rearrange("(t p f) c -> t p (f c)", t=T, p=P)
    outv = out[:].rearrange("(t p f) c -> t p (f c)", t=T, p=P)

    with tile.TileContext(nc) as tc:
        with (
            tc.tile_pool(name="pin", bufs=2) as pin,
            tc.tile_pool(name="pout", bufs=2) as pout,
            tc.tile_pool(name="pch", bufs=1) as pch,
            tc.tile_pool(name="pch2", bufs=2) as pch2,
            tc.tile_pool(name="pmat", bufs=1) as pmat,
            tc.tile_pool(name="pmat2", bufs=2) as pmat2,
            tc.tile_pool(name="pc", bufs=1) as pc,
        ):
            pi2 = pc.tile([P, 1], F32, tag="pi2")
            nc.vector.memset(pi2[:], PI / 2)

            def rotation(v3, pref):
                """v3: [P, 3F] f32 interleaved rotation vectors ->
                dict {(i,j): fp16 [P,F] plane} of the rotation matrix."""
                vv = v3[:].rearrange("p (f c) -> p c f", c=3)
                sq = pch2.tile([P, 3 * F], F32, tag=pref + "sq")
                nc.scalar.activation(sq[:], v3[:], AF.Square)
                sqv = sq[:].rearrange("p (f c) -> p c f", c=3)
                th2a = pch.tile([P, F], F32, tag=pref + "th2a")
                nc.vector.tensor_add(th2a[:], sqv[:, 0, :], sqv[:, 1, :])
                th2 = pch.tile([P, F], F32, tag=pref + "th2")
                nc.vector.tensor_add(th2[:], th2a[:], sqv[:, 2, :])
                lg = pch2.tile([P, F], F32, tag=pref + "lg")
                nc.scalar.activation(lg[:], th2[:], AF.Ln)
                th = pch2.tile([P, F], F32, tag=pref + "th")
                nc.scalar.activation(th[:], lg[:], AF.Exp, scale=0.5)
                rth = pch2.tile([P, F], F32, tag=pref + "rth")
                nc.scalar.activation(rth[:], lg[:], AF.Exp, scale=-0.5)
                # sh = sin(th/2); dataset max |v| ~ 5.6 < 2*pi so th/2 < pi.
                sh = pch2.tile([P, F], F32, tag=pref + "sh")
                nc.scalar.activation(sh[:], th[:], AF.Sin, scale=0.5)
                # ch = cos(th/2) = sin(th/2 + pi/2), wrapped when th > pi
                m = pch.tile([P, F], F32, tag=pref + "m")
                nc.vector.tensor_scalar(m[:], th[:], PI, None, OP.is_gt)
                u4 = pch.tile([P, F], F32, tag=pref + "u4")
                nc.vector.scalar_tensor_tensor(
                    u4[:], m[:], -4 * PI, th[:], OP.mult, OP.add
                )
                ch = pch2.tile([P, F], F32, tag=pref + "ch")
                nc.scalar.activation(ch[:], u4[:], AF.Sin, scale=0.5, bias=pi2[:])
                # t = sh * rth;  a = 2*t*ch;  b = 2*t^2;  c = 1 - 2*sh^2
                t = pch.tile([P, F], F32, tag=pref + "t")
                nc.vector.tensor_mul(t[:], sh[:], rth[:])
                a16 = pmat2.tile([P, F], mat_dt, tag=pref + "a16")
                nc.vector.scalar_tensor_tensor(
                    a16[:], t[:], 2.0, ch[:], OP.mult, OP.mult
                )
                b16 = pmat2.tile([P, F], mat_dt, tag=pref + "b16")
                nc.scalar.activation(b16[:], t[:], AF.Square, scale=math.sqrt(2.0))
                sh2d = pch.tile([P, F], F32, tag=pref + "sh2d")
                nc.scalar.activation(sh2d[:], sh[:], AF.Square, scale=math.sqrt(2.0))
                c16 = pmat2.tile([P, F], mat_dt, tag=pref + "c16")
                nc.scalar.activation(c16[:], sh2d[:], AF.Identity, scale=-1.0, bias=1.0)
                # deinterleave v -> fp16 planes
                vh = []
                for ci, cn in enumerate("xyz"):
                    vt = pmat2.tile([P, F], mat_dt, tag=pref + "v" + cn, name=pref + "v" + cn)
                    nc.scalar.activation(vt[:], vv[:, ci, :], AF.Copy)
                    vh.append(vt)
                # matrix build (fp16, 2x mode)
                def mk(tag):
                    return pmat.tile([P, F], mat_dt, tag=pref + tag, name=pref + tag)
                bv, av = [], []
                for ci, cn in enumerate("xyz"):
                    bt = mk("b" + cn)
                    nc.vector.tensor_mul(bt[:], b16[:], vh[ci][:])
                    bv.append(bt)
                    at = mk("a" + cn)
                    nc.vector.tensor_mul(at[:], a16[:], vh[ci][:])
                    av.append(at)
                R = {}
                # diagonal: R_ii = c + b*v_i^2
                for i in range(3):
                    d = mk(f"d{i}")
                    nc.vector.tensor_mul(d[:], bv[i][:], vh[i][:])
                    r = mk(f"R{i}{i}")
                    nc.vector.tensor_add(r[:], d[:], c16[:])
                    R[(i, i)] = r
                # off-diagonal pairs
                for (i, j, k) in ((0, 1, 2), (0, 2, 1), (1, 2, 0)):
                    pij = mk(f"p{i}{j}")
                    nc.vector.tensor_mul(pij[:], bv[i][:], vh[j][:])
                    rij = mk(f"R{i}{j}")
                    rji = mk(f"R{j}{i}")
                    # R[i][j] = b vi vj + a * skew term
                    # skew: R01=-az R10=+az R02=+ay R20=-ay R12=-ax R21=+ax
                    if (i, j) == (0, 2):
                        nc.vector.tensor_add(rij[:], pij[:], av[k][:])
                        nc.vector.tensor_sub(rji[:], pij[:], av[k][:])
                    else:
                        nc.vector.tensor_sub(rij[:], pij[:], av[k][:])
                        nc.vector.tensor_add(rji[:], pij[:], av[k][:])
                    R[(i, j)] = rij
                    R[(j, i)] = rji
                return R

            def emit_tile(ti):
                r3 = pin.tile([P, 3 * F], F32, tag="rot3")
                nc.sync.dma_start(out=r3[:], in_=rotv[ti])
                s3 = pin.tile([P, 3 * F], F32, tag="sd3")
                nc.sync.dma_start(out=s3[:], in_=sdv[ti])
                c3 = pin.tile([P, 3 * F], F32, tag="sc3")
                nc.sync.dma_start(out=c3[:], in_=scv[ti])
                t3 = pin.tile([P, 3 * F], F32, tag="tr3")
                nc.sync.dma_start(out=t3[:], in_=trv[ti])
                ot = pout.tile([P, 12 * F], F32, tag="out")
                otv = ot[:].rearrange("p (f c) -> p c f", c=12)

                Rm = rotation(r3, "R")
                Um = rotation(s3, "U")

                # e_k = exp(scal_k / 2) fp16 planes
                scv3 = c3[:].rearrange("p (f c) -> p c f", c=3)
                eh = []
                for k in range(3):
                    ek = pmat2.tile([P, F], MAT_DT, tag=f"e{k}", name=f"e{k}")
                    nc.scalar.activation(ek[:], scv3[:, k, :], AF.Exp, scale=0.5)
                    eh.append(ek)
                # W[i][k] = U[i][k] * e_k
                W = {}
                for i in range(3):
                    for k in range(3):
                        w = pmat.tile([P, F], MAT_DT, tag=f"W{i}{k}", name=f"W{i}{k}")
                        nc.vector.tensor_mul(w[:], Um[(i, k)][:], eh[k][:])
                        W[(i, k)] = w
                # S = W @ W^T (symmetric, 6 unique)
                S = {}
                for i in range(3):
                    for j in range(i, 3):
                        if i == j and OPT_S_DIAG_ACT:
                            sqs = []
                            for k in range(3):
                                q = pmat.tile([P, F], MAT_DT, tag=f"sq{k}", name=f"sq{k}")
                                nc.scalar.activation(q[:], W[(i, k)][:], AF.Square)
                                sqs.append(q)
                            s12 = pmat.tile([P, F], MAT_DT, tag="sm12", name="sm12")
                            nc.vector.tensor_add(s12[:], sqs[0][:], sqs[1][:])
                            sij = pmat.tile([P, F], MAT_DT, tag=f"S{i}{j}", name=f"S{i}{j}")
                            nc.vector.tensor_add(sij[:], s12[:], sqs[2][:])
                        else:
                            m1 = pmat.tile([P, F], MAT_DT, tag="sm1", name="sm1")
                            nc.vector.tensor_mul(m1[:], W[(i, 0)][:], W[(j, 0)][:])
                            m2 = pmat.tile([P, F], MAT_DT, tag="sm2", name="sm2")
                            nc.vector.tensor_mul(m2[:], W[(i, 1)][:], W[(j, 1)][:])
                            s12 = pmat.tile([P, F], MAT_DT, tag="sm12", name="sm12")
                            nc.vector.tensor_add(s12[:], m1[:], m2[:])
                            m3 = pmat.tile([P, F], MAT_DT, tag="sm3", name="sm3")
                            nc.vector.tensor_mul(m3[:], W[(i, 2)][:], W[(j, 2)][:])
                            sij = pmat.tile([P, F], MAT_DT, tag=f"S{i}{j}", name=f"S{i}{j}")
                            nc.vector.tensor_add(sij[:], s12[:], m3[:])
                        S[(i, j)] = S[(j, i)] = sij
                # M = R @ S -> out cols 4i+j
                for i in range(3):
                    for j in range(3):
                        m1 = pmat.tile([P, F], MAT_DT, tag="mm1", name="mm1")
                        nc.vector.tensor_mul(m1[:], Rm[(i, 0)][:], S[(0, j)][:])
                        m2 = pmat.tile([P, F], MAT_DT, tag="mm2", name="mm2")
                        nc.vector.tensor_mul(m2[:], Rm[(i, 1)][:], S[(1, j)][:])
                        s12 = pmat.tile([P, F], MAT_DT, tag="mm12", name="mm12")
                        nc.vector.tensor_add(s12[:], m1[:], m2[:])
                        m3 = pmat.tile([P, F], MAT_DT, tag="mm3", name="mm3")
                        nc.vector.tensor_mul(m3[:], Rm[(i, 2)][:], S[(2, j)][:])
                        if OPT_M_PLANAR:
                            mij = pmat2.tile([P, F], MAT_DT, tag="mij", name="mij")
                            nc.vector.tensor_add(mij[:], s12[:], m3[:])
                            nc.scalar.activation(
                                otv[:, 4 * i + j, :], mij[:], AF.Copy
                            )
                        else:
                            nc.vector.tensor_add(otv[:, 4 * i + j, :], s12[:], m3[:])
                # trans -> out cols 4k+3
                trv3 = t3[:].rearrange("p (f c) -> p c f", c=3)
                for k in range(3):
                    nc.scalar.activation(
                        otv[:, 4 * k + 3, :], trv3[:, k, :], AF.Copy
                    )
                nc.sync.dma_start(out=outv[ti], in_=ot[:])

            if loop_rep is None:
                for ti_ in range(T):
                    emit_tile(ti_)
            else:
                with tc.For_i(0, loop_rep, 1):
                    emit_tile(0)

    _split_multi_waits(nc)
    return nc


# ----------------------------------------------------------------------------
# host-side execution
# ----------------------------------------------------------------------------
_CACHE = {}


def _get_runner():
    if "runner" in _CACHE:
        return _CACHE["runner"]
    import jax
    from jax.sharding import Mesh, PartitionSpec
    from jax.experimental.shard_map import shard_map
    from concourse.bass2jax import (
        _bass_exec_p,
        install_neuronx_cc_hook,
        partition_id_tensor,
    )

    nc = build_module()
    install_neuronx_cc_hook()
    partition_name = nc.partition_id_tensor.name if nc.partition_id_tensor else None
    in_names, out_names, out_avals, zero_outs = [], [], [], []
    for alloc in nc.m.functions[0].allocations:
        if not isinstance(alloc, mybir.MemoryLocationSet):
            continue
        name = alloc.memorylocations[0].name
        if alloc.kind == "ExternalInput":
            if name != partition_name:
                in_names.append(name)
        elif alloc.kind == "ExternalOutput":
            shape = tuple(alloc.tensor_shape)
            dtype = mybir.dt.np(alloc.dtype)
            out_names.append(name)
            out_avals.append(jax.core.ShapedArray(shape, dtype))
            zero_outs.append(np.zeros(shape, dtype))
    n_params = len(in_names)
    all_in_names = in_names + out_names + (
        [partition_name] if partition_name else []
    )

    def _body(*args):
        operands = list(args)
        if partition_name is not None:
            operands.append(partition_id_tensor())
        outs = _bass_exec_p.bind(
            *operands,
            out_avals=tuple(out_avals),
            in_names=tuple(all_in_names),
            out_names=tuple(out_names),
            lowering_input_output_aliases=(),
            sim_require_finite=True,
            sim_require_nnan=True,
            nc=nc,
        )
        return tuple(outs)

    devices = jax.devices()[:N_CORES]
    mesh = Mesh(np.asarray(devices), ("core",))
    n_outs = len(out_names)
    jf = jax.jit(
        shard_map(
            _body,
            mesh=mesh,
            in_specs=(PartitionSpec("core"),) * (n_params + n_outs),
            out_specs=(PartitionSpec("core"),) * n_outs,
            check_rep=False,
        ),
        donate_argnums=tuple(range(n_params, n_params + n_outs)),
        keep_unused=True,
    )
    _CACHE["runner"] = (jf, in_names, out_names, zero_outs)
    return _CACHE["runner"]


def kernel(trans, rotat, scal_dir, scal):
    jf, in_names, out_names, zero_outs = _get_runner()
    inputs = {"trans": trans, "rotat": rotat, "scal_dir": scal_dir, "scal": scal}
    # pad to BPAD with ones (zeros would make |v| = 0 -> inf/NaN chains)
    padded = {}
    for k, v in inputs.items():
        a = np.ones((BPAD, 3), dtype=np.float32)
        a[:B] = v
        padded[k] = a
    args = [padded[n] for n in in_names]
    zeros = [np.zeros((N_CORES * z.shape[0], *z.shape[1:]), z.dtype) for z in zero_outs]
    outs = jf(*args, *zeros)
    full = np.asarray(outs[0])  # [BPAD, 12]
    return full[:B].reshape(B, 3, 4).astype(np.float32, copy=False)


if __name__ == "__main__":
    rng = np.random.default_rng(0)
    Bt = B
    ins = {
        "trans": rng.normal(size=(Bt, 3)).astype(np.float32),
        "rotat": rng.normal(size=(Bt, 3)).astype(np.float32),
        "scal_dir": rng.normal(size=(Bt, 3)).astype(np.float32),
        "scal": rng.normal(size=(Bt, 3)).astype(np.float32),
    }
    out = kernel(**ins)
    print(out.shape, out.dtype)
